# revision 25
# baseline (speedup 1.0000x reference)
"""Trainium2 Bass kernel for nn_Conv4Pim_group_split_v2 (dense CNN, PIM-style
group-split quantized conv).

Reference computation (B=32, IC=256, H=W=32, OC=256, GROUPS=4, K=3, pad=1):
  for each branch (p: relu(W), n: relu(-W)) with scales (s_w, s_ps[4]):
    w_int = round(clip(relu_w / s_w, 0, 15));  w_arr = (w_int mod 4) * s_w
    conv  = conv2d(x, w_arr)                        # [B, 4*256, 32, 32]
    per group g: q_g = round(clip(conv_g / s_ps[g], -128, 127)) * s_ps[g]
    branch_out = sum_g q_g                           # [B, 256, 32, 32]
  out = branch_p - branch_n

Kernel strategy:
  - Data-parallel over batch: 8 cores x 4 images, no collectives.
  - Weight quantization done host-side (tiny); device weights are the
    INTEGER values {0,1,2,3} stored in bf16 (exact). The weight scale is
    folded into the psum-quantizer scale alpha = s_w / s_ps.
  - x is split host-side into bf16 hi + lo (x ~= hi + lo, ~16-bit mantissa)
    and the conv runs as 2 accumulating bf16 matmul passes -> near-fp32
    conv accuracy, which matters because the psum quantizer rounds.
  - Conv = 9-offset (3x3) x 2 ic-tile x 2 (hi/lo) = 36 accumulated matmuls
    of [K=128, M=128] x [K=128, N=512] per psum tile, reading a padded
    [128, 34, 34] image held in SBUF.
  - Psum quantize on ACT+DVE: t = psum * alpha (ACT); round via the
    +/- 1.5*2^23 magic trick (DVE, exact RNE like jnp.round); clip to
    [-128,127] (DVE); multiply by +/-s_ps and accumulate group sums (DVE).
"""

import time

import numpy as np
import ml_dtypes
from contextlib import ExitStack

import concourse.bass as bass
import concourse.tile as tile
from concourse import bacc, mybir
from concourse.bass_utils import run_bass_kernel_spmd

dt = mybir.dt
Alu = mybir.AluOpType
AF = mybir.ActivationFunctionType

N_CORES = 8
B, IC, H, W = 32, 256, 32, 32
OC, KS, GROUPS = 256, 3, 4
BPC = B // N_CORES          # batches per core
HP, WP = H + 2, W + 2       # padded image
N_OCT = 16                  # 2048 conv output channels / 128
ROWS_PER_NT = 16            # output rows per psum tile (16*32 = 512 = N)
MAGIC = float(3 * 2**22)    # 1.5*2^23: fp32 RNE rounding constant

_CACHE: dict = {}


def _build_body(ctx: ExitStack, tc, xins, wq, sc, out, n_batches: int,
                n_oct: int, n_iters: int = 1, mode: str = "bf16x2"):
    """Emit the per-core program.

    mode "bf16x2": xins = (xh, xl) bf16 hi/lo DRAM pair, wq bf16.
    mode "fp32r":  xins = (xf,) float32r DRAM, wq float32r, single pass.
    wq:  [128, n_oct*2*9*128] (integer weights, icp-partition)
    sc:  [128, 16] f32 DRAM (col j: alpha_j, col 8+j: beta_j)
    out: [n_batches, 256, 1024] f32 DRAM
    """
    nc = tc.nc
    n_j = n_oct // 2          # number of (branch,group) psum slabs
    n_tout = 2                # output oc tiles (256 oc)
    n_hl = 2 if mode == "bf16x2" else 1
    xdt = dt.bfloat16 if mode == "bf16x2" else dt.float32
    fp32r = mode == "fp32r"
    n_mm = 2 * KS * KS * n_hl

    wpool = ctx.enter_context(tc.tile_pool(name="w", bufs=1))
    spool = ctx.enter_context(tc.tile_pool(name="s", bufs=1))
    xpool = ctx.enter_context(tc.tile_pool(name="x", bufs=2))
    ppool = ctx.enter_context(tc.tile_pool(name="ps", bufs=8, space="PSUM"))
    tpool = ctx.enter_context(tc.tile_pool(name="t", bufs=2))
    apool = ctx.enter_context(tc.tile_pool(name="a", bufs=3))

    sct = spool.tile([128, 16], dt.float32, name="sct")
    nc.sync.dma_start(sct[:], sc[:])

    wdt = dt.float32r if fp32r else xdt
    chunk = 2 * 9 * 128
    # 16 separate weight-chunk tiles: matmuls for output tile `ot` only
    # depend on chunk `ot`, so the PE can start as soon as the first image
    # and chunk 0 have landed instead of waiting for the full 19MB load.
    wts = [wpool.tile([128, chunk], wdt, name=f"wt{ot}") for ot in range(n_oct)]

    def load_x(b):
        xt = {}
        for ict in range(2):
            for hl in range(n_hl):
                tile_dt = dt.float32r if fp32r else xdt
                t = xpool.tile([128, HP, WP], tile_dt,
                               name=f"xp{ict}{hl}", tag=f"xp{ict}{hl}")
                if fp32r:
                    nc.gpsimd.memset(t.bitcast(dt.uint32), 0)
                else:
                    nc.gpsimd.memset(t[:], 0.0)
                nc.sync.dma_start(t[:, 1:H + 1, 1:W + 1], xins[hl][b, ict])
                xt[ict, hl] = t
        return xt

    for ot in range(n_oct):
        nc.sync.dma_start(wts[ot][:], wq[:, ot * chunk:(ot + 1) * chunk])

    loop_ctx = tc.For_i(0, n_iters, 1) if n_iters > 1 else None
    if loop_ctx is not None:
        ctx.enter_context(loop_ctx)

    for b in range(n_batches):
        xt = load_x(b)

        for nt in range(H // ROWS_PER_NT):
            y0 = nt * ROWS_PER_NT
            for tout in range(n_tout):
                acc = apool.tile([128, 512], dt.float32, name="acc", tag="acc")
                for j in range(n_j):
                    ot = 2 * j + tout
                    ps = ppool.tile([128, 512], dt.float32, name="ps", tag="ps")
                    mm = 0
                    for ict in range(2):
                        for ky in range(KS):
                            for kx in range(KS):
                                for hl in range(n_hl):
                                    base = ((ict * 3 + ky) * 3 + kx) * 128
                                    lhsT = wts[ot][:, base:base + 128]
                                    rhs = xt[ict, hl][:, y0 + ky:y0 + ky + ROWS_PER_NT,
                                                      kx:kx + W]
                                    nc.tensor.matmul(ps[:], lhsT, rhs,
                                                     start=(mm == 0),
                                                     stop=(mm == n_mm - 1))
                                    mm += 1
                    # quantize: round(clip(ps*alpha, -128, 127)) * beta, accumulate.
                    # Round via the 1.5*2^23 magic constant: ACT computes
                    # ps*alpha + MAGIC (fp32 -> forced RNE to integer), DVE
                    # subtracts it back, then clip and scale by +/-s_ps.
                    t1 = tpool.tile([128, 512], dt.float32, name="t1", tag="t1")
                    nc.scalar.activation(t1[:], ps[:], AF.Copy,
                                         scale=sct[:, j:j + 1], bias=MAGIC)
                    t2 = tpool.tile([128, 512], dt.float32, name="t2", tag="t2")
                    nc.vector.tensor_scalar(t2[:], t1[:], MAGIC, -128.0,
                                            Alu.subtract, Alu.max)
                    if j == 0:
                        nc.vector.tensor_scalar(acc[:], t2[:], 127.0,
                                                sct[:, 8 + j:9 + j],
                                                Alu.min, Alu.mult)
                    else:
                        t3 = tpool.tile([128, 512], dt.float32, name="t3", tag="t3")
                        nc.vector.tensor_scalar(t3[:], t2[:], 127.0,
                                                sct[:, 8 + j:9 + j],
                                                Alu.min, Alu.mult)
                        nc.vector.tensor_add(acc[:], acc[:], t3[:])
                nc.sync.dma_start(
                    out[b, 128 * tout:128 * (tout + 1), 512 * nt:512 * (nt + 1)],
                    acc[:])


def _build_body_wino(ctx: ExitStack, tc, xw, uw, sc, out, n_batches: int,
                     n_iters: int = 1):
    """Winograd F(2x2,3x3) conv: 2.25x fewer PE columns than direct.

    U = GwG^T (exact in bf16: quarter-integers <= 6.75), V = B^T d B built on
    DVE in fp32 then stored bf16; M[plane] = sum_ict U^T V on the PE (32 MMs
    of [128,128]x[128,256] per (ocb, img)); inverse transform A^T M A on
    DVE/ACT/GPSIMD; then the usual psum-quantizer chain per ocb=(tout,j).

    xw: [n_b, 2, 128, 2, 16, 2, 16] f32  (rows/cols split into parity pairs)
    uw: [16, 128, 2*16*128] bf16         (ocb, icp, ict*plane*oci)
    sc: [128, 16] f32 (col j: alpha_j, col 8+j: beta_j)
    out: [n_b, 256, 1024] f32
    """
    nc = tc.nc
    f32, bf16 = dt.float32, dt.bfloat16

    spool = ctx.enter_context(tc.tile_pool(name="s", bufs=1))
    xpool = ctx.enter_context(tc.tile_pool(name="x", bufs=2))
    rpool = ctx.enter_context(tc.tile_pool(name="r", bufs=1))
    vpool = ctx.enter_context(tc.tile_pool(name="v", bufs=1))
    upool = ctx.enter_context(tc.tile_pool(name="u", bufs=2))
    ppool = ctx.enter_context(tc.tile_pool(name="ps", bufs=2, space="PSUM"))
    zpool = ctx.enter_context(tc.tile_pool(name="z", bufs=1))
    cpool = ctx.enter_context(tc.tile_pool(name="c", bufs=2))
    tpool = ctx.enter_context(tc.tile_pool(name="t", bufs=2))
    apool = ctx.enter_context(tc.tile_pool(name="a", bufs=1))

    sct = spool.tile([128, 16], f32, name="sct")
    nc.sync.dma_start(sct[:], sc[:])

    loop_ctx = tc.For_i(0, n_iters, 1) if n_iters > 1 else None
    if loop_ctx is not None:
        ctx.enter_context(loop_ctx)

    # V for all images resident: [128, img, ict, plane, t, u] bf16 (64KB/par)
    vt = vpool.tile([128, n_batches, 2, 16, 16, 16], bf16, name="vt")
    acc = apool.tile([128, n_batches, 2, 1024], f32, name="acc")

    # ---- input transform ----
    for b in range(n_batches):
        # parity-plane padded image: [ict, rowpar, 17a, colpar, 17c]:
        # padded pixel (2a+rp, 2c+cp) lives at [ict, rp, a, cp, c].
        # (cp, c) innermost keeps every transform AP at <= 2 free dims.
        xt = xpool.tile([128, 2, 2, 17, 2, 17], f32, name="xt", tag="xt")
        nc.gpsimd.memset(xt[:], 0.0)
        for ict in range(2):
            for sr in range(2):
                for scp in range(2):
                    # source row 2q+sr -> padded row 2q+sr+1:
                    # sr=0 -> (rp=1, a=q); sr=1 -> (rp=0, a=q+1)
                    ra = slice(0, 16) if sr == 0 else slice(1, 17)
                    ca = slice(0, 16) if scp == 0 else slice(1, 17)
                    nc.sync.dma_start(
                        xt[:, ict, 1 - sr, ra, 1 - scp, ca],
                        xw[b, ict, :, sr, :, scp, :])
        rt = rpool.tile([128, 2, 4, 16, 2, 17], f32, name="rt", tag="rt")
        # padded row 2t+i: i=0:(rp0,a=t) 1:(rp1,t) 2:(rp0,t+1) 3:(rp1,t+1)
        for ict in range(2):
            r0 = xt[:, ict, 0, 0:16, :, :]
            r1 = xt[:, ict, 1, 0:16, :, :]
            r2 = xt[:, ict, 0, 1:17, :, :]
            r3 = xt[:, ict, 1, 1:17, :, :]
            nc.vector.tensor_sub(rt[:, ict, 0], r0, r2)
            nc.vector.tensor_add(rt[:, ict, 1], r1, r2)
            nc.vector.tensor_sub(rt[:, ict, 2], r2, r1)
            nc.vector.tensor_sub(rt[:, ict, 3], r1, r3)
        for ict in range(2):
            for xi in range(4):
                c0 = rt[:, ict, xi, :, 0, 0:16]
                c1 = rt[:, ict, xi, :, 1, 0:16]
                c2 = rt[:, ict, xi, :, 0, 1:17]
                c3 = rt[:, ict, xi, :, 1, 1:17]
                eng = nc.vector if xi < 2 else nc.gpsimd
                eng.tensor_sub(vt[:, b, ict, 4 * xi + 0], c0, c2)
                eng.tensor_add(vt[:, b, ict, 4 * xi + 1], c1, c2)
                eng.tensor_sub(vt[:, b, ict, 4 * xi + 2], c2, c1)
                eng.tensor_sub(vt[:, b, ict, 4 * xi + 3], c1, c3)

    # ---- matmuls + inverse transform + quantize, ocb = tout*8 + j ----
    # Output free layout is (p, t, u, q) p-major — the host unpermutes to
    # spatial row-major.  Quantizer runs batched over image PAIRS to halve
    # the elementwise op count (per-op dispatch overhead dominates at 1K).
    for ocb in range(16):
        tout, j = divmod(ocb, 8)
        ut = upool.tile([128, 2, 16, 128], bf16, name="ut", tag="ut")
        nc.sync.dma_start(ut[:], uw[ocb])
        for bp in range(n_batches // 2):
            bsl = slice(2 * bp, 2 * bp + 2)
            cv = cpool.tile([128, 2, 2, 2, 16, 16], bf16, name="cv", tag="cv")
            # both images of the pair share each matmul: rhs [128, 2, 16, 16]
            # = N=512, halving MM and LDWEIGHTS count.  PSUM in 4-plane
            # quarters (4 banks), quarter q holds M[xi=q][nu, img, t, u].
            ms = {}
            # TensorTensor cannot read PSUM: extract M quarters to SBUF
            # (bf16 -- exact enough after the quantizer, 2x DVE rate) via
            # per-bank tensor_copy (DVE) / activation-copy (ACT), then do
            # all combining on SBUF with big batched ops.
            msb = zpool.tile([128, 4, 4, 2, 16, 16], bf16, name="msb",
                             tag="msb")
            zt = zpool.tile([128, 2, 4, 2, 16, 16], bf16, name="zt",
                            tag="zt")
            for q in range(4):
                pst = ppool.tile([128, 4, 2, 16, 16], f32, name="ps",
                                 tag="ps")
                ms[q] = pst
                for pl in range(4):
                    plane = q * 4 + pl
                    for ict in range(2):
                        nc.tensor.matmul(pst[:, pl],
                                         ut[:, ict, plane, :],
                                         vt[:, bsl, ict, plane],
                                         start=(ict == 0),
                                         stop=(ict == 1))
                for nu in range(4):
                    if q % 2 == 0:
                        nc.vector.tensor_copy(msb[:, q, nu], pst[:, nu])
                    else:
                        nc.scalar.activation(msb[:, q, nu], pst[:, nu],
                                             AF.Copy)
            # rows: Z0 = M0+M1+M2, Z1 = M1-M2-M3 (SBUF, batched over nu,img)
            nc.vector.tensor_add(zt[:, 0], msb[:, 0], msb[:, 1])
            nc.vector.tensor_add(zt[:, 0], zt[:, 0], msb[:, 2])
            nc.vector.tensor_sub(zt[:, 1], msb[:, 1], msb[:, 2])
            nc.vector.tensor_sub(zt[:, 1], zt[:, 1], msb[:, 3])
            # cols (batched over img): q0 = Z0+Z1+Z2, q1 = Z1-Z2-Z3 per p
            for p in range(2):
                o0 = cv[:, :, 0, p]
                o1 = cv[:, :, 1, p]
                nc.gpsimd.tensor_add(o0, zt[:, p, 0], zt[:, p, 1])
                nc.gpsimd.tensor_add(o0, o0, zt[:, p, 2])
                nc.gpsimd.tensor_sub(o1, zt[:, p, 1], zt[:, p, 2])
                nc.gpsimd.tensor_sub(o1, o1, zt[:, p, 3])
            # quantize pair: round(clip(c*alpha,-128,127))*beta, accumulate
            t1 = tpool.tile([128, 2048], f32, name="t1", tag="t1")
            nc.scalar.activation(t1[:], cv[:], AF.Copy,
                                 scale=sct[:, j:j + 1], bias=MAGIC)
            nc.vector.tensor_scalar(t1[:], t1[:], MAGIC, -128.0,
                                    Alu.subtract, Alu.max)
            asl = acc[:, 2 * bp:2 * bp + 2, tout, :]
            if j == 0:
                nc.vector.tensor_scalar(asl, t1[:], 127.0,
                                        sct[:, 8 + j:9 + j],
                                        Alu.min, Alu.mult)
            else:
                nc.vector.tensor_scalar(t1[:], t1[:], 127.0,
                                        sct[:, 8 + j:9 + j],
                                        Alu.min, Alu.mult)
                nc.gpsimd.tensor_add(asl, asl, t1[:])
            if j == 7:
                for bi in range(2):
                    b = 2 * bp + bi
                    nc.sync.dma_start(
                        out[b, 128 * tout:128 * (tout + 1), :],
                        acc[:, b, tout, :])


def build_program(n_batches: int = BPC, n_oct: int = N_OCT, n_iters: int = 1,
                  mode: str = "bf16x2"):
    if mode == "wino":
        nc = bacc.Bacc("TRN2", target_bir_lowering=False, debug=False,
                       enable_asserts=False, num_devices=N_CORES)
        xw = nc.dram_tensor("xw", [n_batches, 2, 128, 2, 16, 2, 16],
                            dt.float32, kind="ExternalInput").ap()
        uw = nc.dram_tensor("uw", [16, 128, 2 * 16 * 128], dt.bfloat16,
                            kind="ExternalInput").ap()
        sc = nc.dram_tensor("sc", [128, 16], dt.float32,
                            kind="ExternalInput").ap()
        out = nc.dram_tensor("out", [n_batches, 256, H * W], dt.float32,
                             kind="ExternalOutput").ap()
        with tile.TileContext(nc) as tc, ExitStack() as ctx:
            _build_body_wino(ctx, tc, xw, uw, sc, out, n_batches, n_iters)
        nc.compile()
        return nc
    nc = bacc.Bacc("TRN2", target_bir_lowering=False, debug=False,
                   enable_asserts=False, num_devices=N_CORES)
    xdt = dt.bfloat16 if mode == "bf16x2" else dt.float32r
    if mode == "bf16x2":
        xins = (nc.dram_tensor("xh", [n_batches, 2, 128, H, W], xdt,
                               kind="ExternalInput").ap(),
                nc.dram_tensor("xl", [n_batches, 2, 128, H, W], xdt,
                               kind="ExternalInput").ap())
    else:
        xins = (nc.dram_tensor("xf", [n_batches, 2, 128, H, W], xdt,
                               kind="ExternalInput").ap(),)
    wq = nc.dram_tensor("wq", [128, n_oct * 2 * 9 * 128], xdt,
                        kind="ExternalInput").ap()
    sc = nc.dram_tensor("sc", [128, 16], dt.float32,
                        kind="ExternalInput").ap()
    out = nc.dram_tensor("out", [n_batches, 256, H * W], dt.float32,
                         kind="ExternalOutput").ap()
    with tile.TileContext(nc) as tc, ExitStack() as ctx:
        _build_body(ctx, tc, xins, wq, sc, out, n_batches, n_oct, n_iters, mode)
    nc.compile()
    return nc


def _quant_weights(weight: np.ndarray, s_w: np.float32) -> np.ndarray:
    """(round(clip(relu_w / s_w, 0, 15)) mod 4) as float32 integers."""
    w = weight.astype(np.float32)
    w_int = np.round(np.clip(w / np.float32(s_w), np.float32(0.0),
                             np.float32(15.0)))
    return np.mod(w_int, np.float32(4.0))


_G_WINO = np.array([[1, 0, 0], [0.5, 0.5, 0.5], [0.5, -0.5, 0.5], [0, 0, 1]],
                   np.float32)


def prep_inputs(x, weight, s_w_p, s_w_n, s_ps_p, s_ps_n, mode: str = "bf16x2"):
    """Host-side prep: returns (in_maps list of 8 dicts)."""
    x = np.asarray(x, dtype=np.float32)
    weight = np.asarray(weight, dtype=np.float32)
    s_w_p = np.float32(np.asarray(s_w_p).reshape(-1)[0])
    s_w_n = np.float32(np.asarray(s_w_n).reshape(-1)[0])
    s_ps_p = np.asarray(s_ps_p, dtype=np.float32).reshape(GROUPS)
    s_ps_n = np.asarray(s_ps_n, dtype=np.float32).reshape(GROUPS)

    if mode == "wino":
        wq_p = _quant_weights(np.maximum(weight, 0.0), s_w_p)
        wq_n = _quant_weights(np.maximum(-weight, 0.0), s_w_n)
        w_all = np.concatenate([wq_p, wq_n], axis=0)    # [2048, 256, 3, 3]
        # U[oc, ic, xi, nu] = G w G^T  (exact quarter-integers <= 6.75)
        U = np.einsum("xi,acij,nj->acxn", _G_WINO, w_all, _G_WINO,
                      optimize=True).astype(np.float32)
        # -> uw[ocb = tout*8 + j][icp][ict, plane, oci]
        U = U.reshape(8, 2, 128, 2, 128, 16)            # j,tout,oci,ict,icp,pl
        uw = (U.transpose(1, 0, 4, 3, 5, 2)             # tout,j,icp,ict,pl,oci
              .reshape(16, 128, 2 * 16 * 128)
              .astype(ml_dtypes.bfloat16))
        uw = np.ascontiguousarray(uw)

        alpha = np.concatenate([s_w_p / s_ps_p, s_w_n / s_ps_n]).astype(np.float32)
        beta = np.concatenate([s_ps_p, -s_ps_n]).astype(np.float32)
        row = np.zeros(16, dtype=np.float32)
        row[0:8] = alpha
        row[8:16] = beta
        sc_np = np.ascontiguousarray(np.tile(row, (128, 1)))

        xs = x.reshape(B, 2, 128, H, W)
        # rows/cols split by parity: [B,2,128, 2(sr),16, 2(sc),16]
        xw = (xs.reshape(B, 2, 128, 16, 2, 16, 2)
              .transpose(0, 1, 2, 4, 3, 6, 5))
        in_maps = []
        for c in range(N_CORES):
            sl = slice(c * BPC, (c + 1) * BPC)
            in_maps.append({"uw": uw, "sc": sc_np,
                            "xw": np.ascontiguousarray(xw[sl])})
        return in_maps

    wq_p = _quant_weights(np.maximum(weight, 0.0), s_w_p)
    wq_n = _quant_weights(np.maximum(-weight, 0.0), s_w_n)
    w_all = np.concatenate([wq_p, wq_n], axis=0)        # [2048, 256, 3, 3]
    wdt = ml_dtypes.bfloat16 if mode == "bf16x2" else np.float32
    # -> wsb[icp, ot, ict, ky, kx, oci]
    wsb = (w_all.reshape(N_OCT, 128, 2, 128, KS, KS)
           .transpose(3, 0, 2, 4, 5, 1)
           .reshape(128, N_OCT * 2 * 9 * 128)
           .astype(wdt))

    # scales table: col j = alpha_j = s_w/s_ps_j ; col 8+j = beta_j = +/-s_ps_j
    alpha = np.concatenate([s_w_p / s_ps_p, s_w_n / s_ps_n]).astype(np.float32)
    beta = np.concatenate([s_ps_p, -s_ps_n]).astype(np.float32)
    row = np.zeros(16, dtype=np.float32)
    row[0:8] = alpha
    row[8:16] = beta
    sc_np = np.ascontiguousarray(np.tile(row, (128, 1)))

    xs = x.reshape(B, 2, 128, H, W)
    if mode == "bf16x2":
        x_hi = xs.astype(ml_dtypes.bfloat16)
        x_lo = (xs - x_hi.astype(np.float32)).astype(ml_dtypes.bfloat16)
    else:
        x_hi = xs  # float32, fed directly as float32r
        x_lo = None

    in_maps = []
    for c in range(N_CORES):
        sl = slice(c * BPC, (c + 1) * BPC)
        m = {"wq": wsb, "sc": sc_np}
        if mode == "bf16x2":
            m["xh"] = np.ascontiguousarray(x_hi[sl])
            m["xl"] = np.ascontiguousarray(x_lo[sl])
        else:
            m["xf"] = np.ascontiguousarray(x_hi[sl])
        in_maps.append(m)
    return in_maps


MODE = "wino"


def kernel(x, weight, s_w_p, s_w_n, s_ps_p, s_ps_n):
    if "nc" not in _CACHE:
        _CACHE["nc"] = build_program(mode=MODE)
    nc = _CACHE["nc"]
    in_maps = prep_inputs(x, weight, s_w_p, s_w_n, s_ps_p, s_ps_n, mode=MODE)
    res = run_bass_kernel_spmd(nc, in_maps, core_ids=list(range(N_CORES)))
    outs = [res.results[c]["out"] for c in range(N_CORES)]
    full = np.concatenate(outs, axis=0)
    if MODE == "wino":
        # device free layout is (q, p, t, u); spatial row-major is
        # (2t+p, 2u+q)
        full = (full.reshape(B, OC, 2, 2, 16, 16)
                .transpose(0, 1, 4, 3, 5, 2))
    full = np.ascontiguousarray(full).reshape(B, OC, H, W)
    return full.astype(np.float32)


# ---------------------------------------------------------------------------
# Timing helper (not used by the grading harness; mirrors
# bass2jax.run_bass_via_pjrt's multi-core path but keeps the jitted callable
# so repeated executions can be timed without retrace overhead).
# ---------------------------------------------------------------------------

def _make_runner(nc):
    import jax
    from jax.experimental.shard_map import shard_map
    from jax.sharding import Mesh, PartitionSpec
    from concourse import bass2jax

    bass2jax.install_neuronx_cc_hook()
    partition_name = (nc.partition_id_tensor.name
                      if nc.partition_id_tensor else None)
    in_names, out_names, out_avals = [], [], []
    for alloc in nc.m.functions[0].allocations:
        if not isinstance(alloc, mybir.MemoryLocationSet):
            continue
        name = alloc.memorylocations[0].name
        if alloc.kind == "ExternalInput":
            if name != partition_name:
                in_names.append(name)
        elif alloc.kind == "ExternalOutput":
            out_names.append(name)
            out_avals.append(jax.core.ShapedArray(tuple(alloc.tensor_shape),
                                                  mybir.dt.np(alloc.dtype)))
    n_params = len(in_names)
    all_names = list(in_names) + list(out_names)
    if partition_name is not None:
        all_names.append(partition_name)

    def _body(*args):
        operands = list(args)
        if partition_name is not None:
            operands.append(bass2jax.partition_id_tensor())
        outs = bass2jax._bass_exec_p.bind(
            *operands,
            out_avals=tuple(out_avals),
            in_names=tuple(all_names),
            out_names=tuple(out_names),
            lowering_input_output_aliases=(),
            sim_require_finite=False,
            sim_require_nnan=False,
            nc=nc,
        )
        return tuple(outs)

    devices = jax.devices()[:N_CORES]
    mesh = Mesh(np.asarray(devices), ("core",))
    n_outs = len(out_names)
    in_specs = (PartitionSpec("core"),) * (n_params + n_outs)
    out_specs = (PartitionSpec("core"),) * n_outs
    donate = tuple(range(n_params, n_params + n_outs))
    sharded = jax.jit(
        shard_map(_body, mesh=mesh, in_specs=in_specs, out_specs=out_specs,
                  check_rep=False),
        donate_argnums=donate, keep_unused=True)
    return sharded, in_names, out_names, out_avals


def time_device(inputs, iters: int = 10):
    """Min wall time per execution of the staged, pre-jitted program."""
    import jax
    if "nc" not in _CACHE:
        _CACHE["nc"] = build_program(mode=MODE)
    nc = _CACHE["nc"]
    in_maps = prep_inputs(**inputs, mode=MODE)
    sharded, in_names, out_names, out_avals = _make_runner(nc)
    concat_in = [np.concatenate([in_maps[c][n] for c in range(N_CORES)], axis=0)
                 for n in in_names]
    dev_in = [jax.device_put(a) for a in concat_in]
    zeros = [np.zeros((N_CORES * a.shape[0], *a.shape[1:]), a.dtype)
             for a in out_avals]
    # warmup + correctness of path
    out = sharded(*dev_in, *[jax.device_put(z) for z in zeros])
    jax.block_until_ready(out)
    times = []
    for _ in range(iters):
        zdev = [jax.device_put(z) for z in zeros]
        jax.block_until_ready(zdev)
        t0 = time.monotonic()
        out = sharded(*dev_in, *zdev)
        jax.block_until_ready(out)
        times.append(time.monotonic() - t0)
    return min(times) * 1e9



# revision 26
# speedup vs baseline: 1.5692x; 1.5692x over previous
"""Trainium2 Bass kernel for nn_Conv4Pim_group_split_v2 (dense CNN, PIM-style
group-split quantized conv).

Reference computation (B=32, IC=256, H=W=32, OC=256, GROUPS=4, K=3, pad=1):
  for each branch (p: relu(W), n: relu(-W)) with scales (s_w, s_ps[4]):
    w_int = round(clip(relu_w / s_w, 0, 15));  w_arr = (w_int mod 4) * s_w
    conv  = conv2d(x, w_arr)                        # [B, 4*256, 32, 32]
    per group g: q_g = round(clip(conv_g / s_ps[g], -128, 127)) * s_ps[g]
    branch_out = sum_g q_g                           # [B, 256, 32, 32]
  out = branch_p - branch_n

Kernel strategy:
  - Data-parallel over batch: 8 cores x 4 images, no collectives.
  - Weight quantization done host-side (tiny); device weights are the
    INTEGER values {0,1,2,3} stored in bf16 (exact). The weight scale is
    folded into the psum-quantizer scale alpha = s_w / s_ps.
  - x is split host-side into bf16 hi + lo (x ~= hi + lo, ~16-bit mantissa)
    and the conv runs as 2 accumulating bf16 matmul passes -> near-fp32
    conv accuracy, which matters because the psum quantizer rounds.
  - Conv = 9-offset (3x3) x 2 ic-tile x 2 (hi/lo) = 36 accumulated matmuls
    of [K=128, M=128] x [K=128, N=512] per psum tile, reading a padded
    [128, 34, 34] image held in SBUF.
  - Psum quantize on ACT+DVE: t = psum * alpha (ACT); round via the
    +/- 1.5*2^23 magic trick (DVE, exact RNE like jnp.round); clip to
    [-128,127] (DVE); multiply by +/-s_ps and accumulate group sums (DVE).
"""

import time

import numpy as np
import ml_dtypes
from contextlib import ExitStack

import concourse.bass as bass
import concourse.tile as tile
from concourse import bacc, mybir
from concourse.bass_utils import run_bass_kernel_spmd

dt = mybir.dt
Alu = mybir.AluOpType
AF = mybir.ActivationFunctionType

N_CORES = 8
B, IC, H, W = 32, 256, 32, 32
OC, KS, GROUPS = 256, 3, 4
BPC = B // N_CORES          # batches per core
HP, WP = H + 2, W + 2       # padded image
N_OCT = 16                  # 2048 conv output channels / 128
ROWS_PER_NT = 16            # output rows per psum tile (16*32 = 512 = N)
MAGIC = float(3 * 2**22)    # 1.5*2^23: fp32 RNE rounding constant

_CACHE: dict = {}


def _build_body(ctx: ExitStack, tc, xins, wq, sc, out, n_batches: int,
                n_oct: int, n_iters: int = 1, mode: str = "bf16x2"):
    """Emit the per-core program.

    mode "bf16x2": xins = (xh, xl) bf16 hi/lo DRAM pair, wq bf16.
    mode "fp32r":  xins = (xf,) float32r DRAM, wq float32r, single pass.
    wq:  [128, n_oct*2*9*128] (integer weights, icp-partition)
    sc:  [128, 16] f32 DRAM (col j: alpha_j, col 8+j: beta_j)
    out: [n_batches, 256, 1024] f32 DRAM
    """
    nc = tc.nc
    n_j = n_oct // 2          # number of (branch,group) psum slabs
    n_tout = 2                # output oc tiles (256 oc)
    n_hl = 2 if mode == "bf16x2" else 1
    xdt = dt.bfloat16 if mode == "bf16x2" else dt.float32
    fp32r = mode == "fp32r"
    n_mm = 2 * KS * KS * n_hl

    wpool = ctx.enter_context(tc.tile_pool(name="w", bufs=1))
    spool = ctx.enter_context(tc.tile_pool(name="s", bufs=1))
    xpool = ctx.enter_context(tc.tile_pool(name="x", bufs=2))
    ppool = ctx.enter_context(tc.tile_pool(name="ps", bufs=8, space="PSUM"))
    tpool = ctx.enter_context(tc.tile_pool(name="t", bufs=2))
    apool = ctx.enter_context(tc.tile_pool(name="a", bufs=3))

    sct = spool.tile([128, 16], dt.float32, name="sct")
    nc.sync.dma_start(sct[:], sc[:])

    wdt = dt.float32r if fp32r else xdt
    chunk = 2 * 9 * 128
    # 16 separate weight-chunk tiles: matmuls for output tile `ot` only
    # depend on chunk `ot`, so the PE can start as soon as the first image
    # and chunk 0 have landed instead of waiting for the full 19MB load.
    wts = [wpool.tile([128, chunk], wdt, name=f"wt{ot}") for ot in range(n_oct)]

    def load_x(b):
        xt = {}
        for ict in range(2):
            for hl in range(n_hl):
                tile_dt = dt.float32r if fp32r else xdt
                t = xpool.tile([128, HP, WP], tile_dt,
                               name=f"xp{ict}{hl}", tag=f"xp{ict}{hl}")
                if fp32r:
                    nc.gpsimd.memset(t.bitcast(dt.uint32), 0)
                else:
                    nc.gpsimd.memset(t[:], 0.0)
                nc.sync.dma_start(t[:, 1:H + 1, 1:W + 1], xins[hl][b, ict])
                xt[ict, hl] = t
        return xt

    for ot in range(n_oct):
        nc.sync.dma_start(wts[ot][:], wq[:, ot * chunk:(ot + 1) * chunk])

    loop_ctx = tc.For_i(0, n_iters, 1) if n_iters > 1 else None
    if loop_ctx is not None:
        ctx.enter_context(loop_ctx)

    for b in range(n_batches):
        xt = load_x(b)

        for nt in range(H // ROWS_PER_NT):
            y0 = nt * ROWS_PER_NT
            for tout in range(n_tout):
                acc = apool.tile([128, 512], dt.float32, name="acc", tag="acc")
                for j in range(n_j):
                    ot = 2 * j + tout
                    ps = ppool.tile([128, 512], dt.float32, name="ps", tag="ps")
                    mm = 0
                    for ict in range(2):
                        for ky in range(KS):
                            for kx in range(KS):
                                for hl in range(n_hl):
                                    base = ((ict * 3 + ky) * 3 + kx) * 128
                                    lhsT = wts[ot][:, base:base + 128]
                                    rhs = xt[ict, hl][:, y0 + ky:y0 + ky + ROWS_PER_NT,
                                                      kx:kx + W]
                                    nc.tensor.matmul(ps[:], lhsT, rhs,
                                                     start=(mm == 0),
                                                     stop=(mm == n_mm - 1))
                                    mm += 1
                    # quantize: round(clip(ps*alpha, -128, 127)) * beta, accumulate.
                    # Round via the 1.5*2^23 magic constant: ACT computes
                    # ps*alpha + MAGIC (fp32 -> forced RNE to integer), DVE
                    # subtracts it back, then clip and scale by +/-s_ps.
                    t1 = tpool.tile([128, 512], dt.float32, name="t1", tag="t1")
                    nc.scalar.activation(t1[:], ps[:], AF.Copy,
                                         scale=sct[:, j:j + 1], bias=MAGIC)
                    t2 = tpool.tile([128, 512], dt.float32, name="t2", tag="t2")
                    nc.vector.tensor_scalar(t2[:], t1[:], MAGIC, -128.0,
                                            Alu.subtract, Alu.max)
                    if j == 0:
                        nc.vector.tensor_scalar(acc[:], t2[:], 127.0,
                                                sct[:, 8 + j:9 + j],
                                                Alu.min, Alu.mult)
                    else:
                        t3 = tpool.tile([128, 512], dt.float32, name="t3", tag="t3")
                        nc.vector.tensor_scalar(t3[:], t2[:], 127.0,
                                                sct[:, 8 + j:9 + j],
                                                Alu.min, Alu.mult)
                        nc.vector.tensor_add(acc[:], acc[:], t3[:])
                nc.sync.dma_start(
                    out[b, 128 * tout:128 * (tout + 1), 512 * nt:512 * (nt + 1)],
                    acc[:])


def _build_body_wino(ctx: ExitStack, tc, xw, uw, sc, out, n_batches: int,
                     n_iters: int = 1):
    """Winograd F(2x2,3x3) conv: 2.25x fewer PE columns than direct.

    U = GwG^T (exact in bf16: quarter-integers <= 6.75), V = B^T d B built on
    DVE in fp32 then stored bf16; M[plane] = sum_ict U^T V on the PE (32 MMs
    of [128,128]x[128,256] per (ocb, img)); inverse transform A^T M A on
    DVE/ACT/GPSIMD; then the usual psum-quantizer chain per ocb=(tout,j).

    xw: [n_b, 2, 128, 2, 16, 2, 16] f32  (rows/cols split into parity pairs)
    uw: [16, 128, 2*16*128] bf16         (ocb, icp, ict*plane*oci)
    sc: [128, 16] f32 (col j: alpha_j, col 8+j: beta_j)
    out: [n_b, 256, 1024] f32
    """
    nc = tc.nc
    f32, bf16 = dt.float32, dt.bfloat16

    spool = ctx.enter_context(tc.tile_pool(name="s", bufs=1))
    xpool = ctx.enter_context(tc.tile_pool(name="x", bufs=2))
    rpool = ctx.enter_context(tc.tile_pool(name="r", bufs=1))
    vpool = ctx.enter_context(tc.tile_pool(name="v", bufs=1))
    upool = ctx.enter_context(tc.tile_pool(name="u", bufs=2))
    ppool = ctx.enter_context(tc.tile_pool(name="ps", bufs=2, space="PSUM"))
    zpool = ctx.enter_context(tc.tile_pool(name="z", bufs=1))
    cpool = ctx.enter_context(tc.tile_pool(name="c", bufs=2))
    tpool = ctx.enter_context(tc.tile_pool(name="t", bufs=2))
    apool = ctx.enter_context(tc.tile_pool(name="a", bufs=1))

    sct = spool.tile([128, 16], f32, name="sct")
    nc.sync.dma_start(sct[:], sc[:])

    loop_ctx = tc.For_i(0, n_iters, 1) if n_iters > 1 else None
    if loop_ctx is not None:
        ctx.enter_context(loop_ctx)

    # V for all images resident: [128, img, ict, plane, t, u] bf16 (64KB/par)
    vt = vpool.tile([128, n_batches, 2, 16, 16, 16], bf16, name="vt")
    acc = apool.tile([128, n_batches, 2, 1024], f32, name="acc")

    # ---- input transform ----
    for b in range(n_batches):
        # parity-plane padded image: [ict, rowpar, 17a, colpar, 17c]:
        # padded pixel (2a+rp, 2c+cp) lives at [ict, rp, a, cp, c].
        # (cp, c) innermost keeps every transform AP at <= 2 free dims.
        xt = xpool.tile([128, 2, 2, 17, 2, 17], f32, name="xt", tag="xt")
        nc.gpsimd.memset(xt[:], 0.0)
        for ict in range(2):
            for sr in range(2):
                for scp in range(2):
                    # source row 2q+sr -> padded row 2q+sr+1:
                    # sr=0 -> (rp=1, a=q); sr=1 -> (rp=0, a=q+1)
                    ra = slice(0, 16) if sr == 0 else slice(1, 17)
                    ca = slice(0, 16) if scp == 0 else slice(1, 17)
                    nc.sync.dma_start(
                        xt[:, ict, 1 - sr, ra, 1 - scp, ca],
                        xw[b, ict, :, sr, :, scp, :])
        rt = rpool.tile([128, 2, 4, 16, 2, 17], f32, name="rt", tag="rt")
        # padded row 2t+i: i=0:(rp0,a=t) 1:(rp1,t) 2:(rp0,t+1) 3:(rp1,t+1)
        for ict in range(2):
            r0 = xt[:, ict, 0, 0:16, :, :]
            r1 = xt[:, ict, 1, 0:16, :, :]
            r2 = xt[:, ict, 0, 1:17, :, :]
            r3 = xt[:, ict, 1, 1:17, :, :]
            nc.vector.tensor_sub(rt[:, ict, 0], r0, r2)
            nc.vector.tensor_add(rt[:, ict, 1], r1, r2)
            nc.vector.tensor_sub(rt[:, ict, 2], r2, r1)
            nc.vector.tensor_sub(rt[:, ict, 3], r1, r3)
        for ict in range(2):
            for xi in range(4):
                c0 = rt[:, ict, xi, :, 0, 0:16]
                c1 = rt[:, ict, xi, :, 1, 0:16]
                c2 = rt[:, ict, xi, :, 0, 1:17]
                c3 = rt[:, ict, xi, :, 1, 1:17]
                eng = nc.vector if xi < 2 else nc.gpsimd
                eng.tensor_sub(vt[:, b, ict, 4 * xi + 0], c0, c2)
                eng.tensor_add(vt[:, b, ict, 4 * xi + 1], c1, c2)
                eng.tensor_sub(vt[:, b, ict, 4 * xi + 2], c2, c1)
                eng.tensor_sub(vt[:, b, ict, 4 * xi + 3], c1, c3)

    # ---- matmuls + inverse transform + quantize, ocb = tout*8 + j ----
    # Output free layout is (p, t, u, q) p-major — the host unpermutes to
    # spatial row-major.  Quantizer runs batched over image PAIRS to halve
    # the elementwise op count (per-op dispatch overhead dominates at 1K).
    for ocb in range(16):
        tout, j = divmod(ocb, 8)
        ut = upool.tile([128, 2, 16, 128], bf16, name="ut", tag="ut")
        nc.sync.dma_start(ut[:], uw[ocb])
        for bp in range(n_batches // 2):
            bsl = slice(2 * bp, 2 * bp + 2)
            cv = cpool.tile([128, 2, 2, 2, 16, 16], bf16, name="cv", tag="cv")
            # both images of the pair share each matmul: rhs [128, 2, 16, 16]
            # = N=512, halving MM and LDWEIGHTS count.  PSUM in 4-plane
            # quarters (4 banks), quarter q holds M[xi=q][nu, img, t, u].
            ms = {}
            # TensorTensor cannot read PSUM: extract M quarters to SBUF
            # (bf16 -- exact enough after the quantizer, 2x DVE rate) via
            # per-bank tensor_copy (DVE) / activation-copy (ACT), then do
            # all combining on SBUF with big batched ops.
            msb = zpool.tile([128, 4, 4, 2, 16, 16], bf16, name="msb",
                             tag="msb")
            zt = zpool.tile([128, 2, 4, 2, 16, 16], bf16, name="zt",
                            tag="zt")
            for q in range(4):
                pst = ppool.tile([128, 4, 2, 16, 16], f32, name="ps",
                                 tag="ps")
                ms[q] = pst
                for pl in range(4):
                    plane = q * 4 + pl
                    for ict in range(2):
                        nc.tensor.matmul(pst[:, pl],
                                         ut[:, ict, plane, :],
                                         vt[:, bsl, ict, plane],
                                         start=(ict == 0),
                                         stop=(ict == 1))
                for nu in range(4):
                    if q % 2 == 0:
                        nc.vector.tensor_copy(msb[:, q, nu], pst[:, nu])
                    else:
                        nc.scalar.activation(msb[:, q, nu], pst[:, nu],
                                             AF.Copy)
            # rows: Z0 = M0+M1+M2, Z1 = M1-M2-M3 (SBUF, batched over nu,img)
            nc.vector.tensor_add(zt[:, 0], msb[:, 0], msb[:, 1])
            nc.vector.tensor_add(zt[:, 0], zt[:, 0], msb[:, 2])
            nc.vector.tensor_sub(zt[:, 1], msb[:, 1], msb[:, 2])
            nc.vector.tensor_sub(zt[:, 1], zt[:, 1], msb[:, 3])
            # cols (batched over img): q0 = Z0+Z1+Z2, q1 = Z1-Z2-Z3 per p
            for p in range(2):
                o0 = cv[:, :, 0, p]
                o1 = cv[:, :, 1, p]
                nc.gpsimd.tensor_add(o0, zt[:, p, 0], zt[:, p, 1])
                nc.gpsimd.tensor_add(o0, o0, zt[:, p, 2])
                nc.gpsimd.tensor_sub(o1, zt[:, p, 1], zt[:, p, 2])
                nc.gpsimd.tensor_sub(o1, o1, zt[:, p, 3])
            # quantize pair: round(clip(c*alpha,-128,127))*beta, accumulate
            t1 = tpool.tile([128, 2048], f32, name="t1", tag="t1")
            nc.scalar.activation(t1[:], cv[:], AF.Copy,
                                 scale=sct[:, j:j + 1], bias=MAGIC)
            nc.vector.tensor_scalar(t1[:], t1[:], MAGIC, -128.0,
                                    Alu.subtract, Alu.max)
            asl = acc[:, 2 * bp:2 * bp + 2, tout, :]
            if j == 0:
                nc.vector.tensor_scalar(asl, t1[:], 127.0,
                                        sct[:, 8 + j:9 + j],
                                        Alu.min, Alu.mult)
            else:
                nc.vector.tensor_scalar(t1[:], t1[:], 127.0,
                                        sct[:, 8 + j:9 + j],
                                        Alu.min, Alu.mult)
                nc.gpsimd.tensor_add(asl, asl, t1[:])
            if j == 7:
                for bi in range(2):
                    b = 2 * bp + bi
                    nc.sync.dma_start(
                        out[b, 128 * tout:128 * (tout + 1), :],
                        acc[:, b, tout, :])


def build_program(n_batches: int = BPC, n_oct: int = N_OCT, n_iters: int = 1,
                  mode: str = "bf16x2"):
    if mode == "wino":
        nc = bacc.Bacc("TRN2", target_bir_lowering=False, debug=False,
                       enable_asserts=False, num_devices=N_CORES)
        xw = nc.dram_tensor("xw", [n_batches, 2, 128, 2, 16, 2, 16],
                            dt.float32, kind="ExternalInput").ap()
        uw = nc.dram_tensor("uw", [16, 128, 2 * 16 * 128], dt.bfloat16,
                            kind="ExternalInput").ap()
        sc = nc.dram_tensor("sc", [128, 16], dt.float32,
                            kind="ExternalInput").ap()
        out = nc.dram_tensor("out", [n_batches, 256, H * W], dt.float32,
                             kind="ExternalOutput").ap()
        with tile.TileContext(nc) as tc, ExitStack() as ctx:
            _build_body_wino(ctx, tc, xw, uw, sc, out, n_batches, n_iters)
        nc.compile()
        return nc
    nc = bacc.Bacc("TRN2", target_bir_lowering=False, debug=False,
                   enable_asserts=False, num_devices=N_CORES)
    xdt = dt.bfloat16 if mode == "bf16x2" else dt.float32r
    if mode == "bf16x2":
        xins = (nc.dram_tensor("xh", [n_batches, 2, 128, H, W], xdt,
                               kind="ExternalInput").ap(),
                nc.dram_tensor("xl", [n_batches, 2, 128, H, W], xdt,
                               kind="ExternalInput").ap())
    else:
        xins = (nc.dram_tensor("xf", [n_batches, 2, 128, H, W], xdt,
                               kind="ExternalInput").ap(),)
    wq = nc.dram_tensor("wq", [128, n_oct * 2 * 9 * 128], xdt,
                        kind="ExternalInput").ap()
    sc = nc.dram_tensor("sc", [128, 16], dt.float32,
                        kind="ExternalInput").ap()
    out = nc.dram_tensor("out", [n_batches, 256, H * W], dt.float32,
                         kind="ExternalOutput").ap()
    with tile.TileContext(nc) as tc, ExitStack() as ctx:
        _build_body(ctx, tc, xins, wq, sc, out, n_batches, n_oct, n_iters, mode)
    nc.compile()
    return nc


def _quant_weights(weight: np.ndarray, s_w: np.float32) -> np.ndarray:
    """(round(clip(relu_w / s_w, 0, 15)) mod 4) as float32 integers."""
    w = weight.astype(np.float32)
    w_int = np.round(np.clip(w / np.float32(s_w), np.float32(0.0),
                             np.float32(15.0)))
    return np.mod(w_int, np.float32(4.0))


_G_WINO = np.array([[1, 0, 0], [0.5, 0.5, 0.5], [0.5, -0.5, 0.5], [0, 0, 1]],
                   np.float32)


def prep_inputs(x, weight, s_w_p, s_w_n, s_ps_p, s_ps_n, mode: str = "bf16x2"):
    """Host-side prep: returns (in_maps list of 8 dicts)."""
    x = np.asarray(x, dtype=np.float32)
    weight = np.asarray(weight, dtype=np.float32)
    s_w_p = np.float32(np.asarray(s_w_p).reshape(-1)[0])
    s_w_n = np.float32(np.asarray(s_w_n).reshape(-1)[0])
    s_ps_p = np.asarray(s_ps_p, dtype=np.float32).reshape(GROUPS)
    s_ps_n = np.asarray(s_ps_n, dtype=np.float32).reshape(GROUPS)

    if mode == "wino":
        wq_p = _quant_weights(np.maximum(weight, 0.0), s_w_p)
        wq_n = _quant_weights(np.maximum(-weight, 0.0), s_w_n)
        w_all = np.concatenate([wq_p, wq_n], axis=0)    # [2048, 256, 3, 3]
        # U[oc, ic, xi, nu] = G w G^T  (exact quarter-integers <= 6.75)
        U = np.einsum("xi,acij,nj->acxn", _G_WINO, w_all, _G_WINO,
                      optimize=True).astype(np.float32)
        # -> uw[ocb = tout*8 + j][icp][ict, plane, oci]
        U = U.reshape(8, 2, 128, 2, 128, 16)            # j,tout,oci,ict,icp,pl
        uw = (U.transpose(1, 0, 4, 3, 5, 2)             # tout,j,icp,ict,pl,oci
              .reshape(16, 128, 2 * 16 * 128)
              .astype(ml_dtypes.bfloat16))
        uw = np.ascontiguousarray(uw)

        alpha = np.concatenate([s_w_p / s_ps_p, s_w_n / s_ps_n]).astype(np.float32)
        beta = np.concatenate([s_ps_p, -s_ps_n]).astype(np.float32)
        row = np.zeros(16, dtype=np.float32)
        row[0:8] = alpha
        row[8:16] = beta
        sc_np = np.ascontiguousarray(np.tile(row, (128, 1)))

        xs = x.reshape(B, 2, 128, H, W)
        # rows/cols split by parity: [B,2,128, 2(sr),16, 2(sc),16]
        xw = (xs.reshape(B, 2, 128, 16, 2, 16, 2)
              .transpose(0, 1, 2, 4, 3, 6, 5))
        in_maps = []
        for c in range(N_CORES):
            sl = slice(c * BPC, (c + 1) * BPC)
            in_maps.append({"uw": uw, "sc": sc_np,
                            "xw": np.ascontiguousarray(xw[sl])})
        return in_maps

    wq_p = _quant_weights(np.maximum(weight, 0.0), s_w_p)
    wq_n = _quant_weights(np.maximum(-weight, 0.0), s_w_n)
    w_all = np.concatenate([wq_p, wq_n], axis=0)        # [2048, 256, 3, 3]
    wdt = ml_dtypes.bfloat16 if mode == "bf16x2" else np.float32
    # -> wsb[icp, ot, ict, ky, kx, oci]
    wsb = (w_all.reshape(N_OCT, 128, 2, 128, KS, KS)
           .transpose(3, 0, 2, 4, 5, 1)
           .reshape(128, N_OCT * 2 * 9 * 128)
           .astype(wdt))

    # scales table: col j = alpha_j = s_w/s_ps_j ; col 8+j = beta_j = +/-s_ps_j
    alpha = np.concatenate([s_w_p / s_ps_p, s_w_n / s_ps_n]).astype(np.float32)
    beta = np.concatenate([s_ps_p, -s_ps_n]).astype(np.float32)
    row = np.zeros(16, dtype=np.float32)
    row[0:8] = alpha
    row[8:16] = beta
    sc_np = np.ascontiguousarray(np.tile(row, (128, 1)))

    xs = x.reshape(B, 2, 128, H, W)
    if mode == "bf16x2":
        x_hi = xs.astype(ml_dtypes.bfloat16)
        x_lo = (xs - x_hi.astype(np.float32)).astype(ml_dtypes.bfloat16)
    else:
        x_hi = xs  # float32, fed directly as float32r
        x_lo = None

    in_maps = []
    for c in range(N_CORES):
        sl = slice(c * BPC, (c + 1) * BPC)
        m = {"wq": wsb, "sc": sc_np}
        if mode == "bf16x2":
            m["xh"] = np.ascontiguousarray(x_hi[sl])
            m["xl"] = np.ascontiguousarray(x_lo[sl])
        else:
            m["xf"] = np.ascontiguousarray(x_hi[sl])
        in_maps.append(m)
    return in_maps


MODE = "fp32r"


def kernel(x, weight, s_w_p, s_w_n, s_ps_p, s_ps_n):
    if "nc" not in _CACHE:
        _CACHE["nc"] = build_program(mode=MODE)
    nc = _CACHE["nc"]
    in_maps = prep_inputs(x, weight, s_w_p, s_w_n, s_ps_p, s_ps_n, mode=MODE)
    res = run_bass_kernel_spmd(nc, in_maps, core_ids=list(range(N_CORES)))
    outs = [res.results[c]["out"] for c in range(N_CORES)]
    full = np.concatenate(outs, axis=0)
    if MODE == "wino":
        # device free layout is (q, p, t, u); spatial row-major is
        # (2t+p, 2u+q)
        full = (full.reshape(B, OC, 2, 2, 16, 16)
                .transpose(0, 1, 4, 3, 5, 2))
    full = np.ascontiguousarray(full).reshape(B, OC, H, W)
    return full.astype(np.float32)


# ---------------------------------------------------------------------------
# Timing helper (not used by the grading harness; mirrors
# bass2jax.run_bass_via_pjrt's multi-core path but keeps the jitted callable
# so repeated executions can be timed without retrace overhead).
# ---------------------------------------------------------------------------

def _make_runner(nc):
    import jax
    from jax.experimental.shard_map import shard_map
    from jax.sharding import Mesh, PartitionSpec
    from concourse import bass2jax

    bass2jax.install_neuronx_cc_hook()
    partition_name = (nc.partition_id_tensor.name
                      if nc.partition_id_tensor else None)
    in_names, out_names, out_avals = [], [], []
    for alloc in nc.m.functions[0].allocations:
        if not isinstance(alloc, mybir.MemoryLocationSet):
            continue
        name = alloc.memorylocations[0].name
        if alloc.kind == "ExternalInput":
            if name != partition_name:
                in_names.append(name)
        elif alloc.kind == "ExternalOutput":
            out_names.append(name)
            out_avals.append(jax.core.ShapedArray(tuple(alloc.tensor_shape),
                                                  mybir.dt.np(alloc.dtype)))
    n_params = len(in_names)
    all_names = list(in_names) + list(out_names)
    if partition_name is not None:
        all_names.append(partition_name)

    def _body(*args):
        operands = list(args)
        if partition_name is not None:
            operands.append(bass2jax.partition_id_tensor())
        outs = bass2jax._bass_exec_p.bind(
            *operands,
            out_avals=tuple(out_avals),
            in_names=tuple(all_names),
            out_names=tuple(out_names),
            lowering_input_output_aliases=(),
            sim_require_finite=False,
            sim_require_nnan=False,
            nc=nc,
        )
        return tuple(outs)

    devices = jax.devices()[:N_CORES]
    mesh = Mesh(np.asarray(devices), ("core",))
    n_outs = len(out_names)
    in_specs = (PartitionSpec("core"),) * (n_params + n_outs)
    out_specs = (PartitionSpec("core"),) * n_outs
    donate = tuple(range(n_params, n_params + n_outs))
    sharded = jax.jit(
        shard_map(_body, mesh=mesh, in_specs=in_specs, out_specs=out_specs,
                  check_rep=False),
        donate_argnums=donate, keep_unused=True)
    return sharded, in_names, out_names, out_avals


def time_device(inputs, iters: int = 10):
    """Min wall time per execution of the staged, pre-jitted program."""
    import jax
    if "nc" not in _CACHE:
        _CACHE["nc"] = build_program(mode=MODE)
    nc = _CACHE["nc"]
    in_maps = prep_inputs(**inputs, mode=MODE)
    sharded, in_names, out_names, out_avals = _make_runner(nc)
    concat_in = [np.concatenate([in_maps[c][n] for c in range(N_CORES)], axis=0)
                 for n in in_names]
    dev_in = [jax.device_put(a) for a in concat_in]
    zeros = [np.zeros((N_CORES * a.shape[0], *a.shape[1:]), a.dtype)
             for a in out_avals]
    # warmup + correctness of path
    out = sharded(*dev_in, *[jax.device_put(z) for z in zeros])
    jax.block_until_ready(out)
    times = []
    for _ in range(iters):
        zdev = [jax.device_put(z) for z in zeros]
        jax.block_until_ready(zdev)
        t0 = time.monotonic()
        out = sharded(*dev_in, *zdev)
        jax.block_until_ready(out)
        times.append(time.monotonic() - t0)
    return min(times) * 1e9

